# revision 31
# baseline (speedup 1.0000x reference)
"""Trainium2 Bass kernel for multi-head attention (B=4, N=2048, C=256, H=16).

Sharding: 8 cores, each core handles one batch b = core//2 and 8 heads
(half of 16) g = core%2.  Each core computes its 8 heads' attention plus a
partial output projection (its heads' rows of w_proj); the host sums the
two partials per batch and adds b_proj.  x is transposed on the host so
the [channels-on-partitions] layout DMAs straight in (no on-chip
transposes for x).

Per-core on-chip algorithm (all layouts "transposed", channels on
partitions):
  qT/kT (spread layout: head j of a 4-head group occupies partitions
        32j..32j+16) = W^T @ xT, bias fused into the single PSUM->SBUF
        tensor_scalar_add                                     [128, N]
  vT   (compact: head lh at partitions 16lh)                  [128, N]
  v_aug[keys, lh, 0:16] = v, v_aug[keys, lh, 16] = 1          (ones col
        makes the attn@v matmul also produce softmax row-sums)
  S^T  = k_h @ q_h^T   (row-group-packed matmuls, K=16)       [keys, q]
  P^T  = exp(S^T): the 256 exp units ([128,1024] tiles) are split
        ~2:1 between ScalarE (table exp -> bf16) and VectorE (one fused
        tensor_scalar: bits = round(128*log2e*s + (128*127-7.4)) as
        int16, whose bit pattern IS bf16(exp(s)) -- Schraudolph approx,
        rel err rms ~1.8%; consumed via a free bitcast).  ScalarE alone
        is a ~290us serial bottleneck; the split takes exp off the
        critical path (final rel_fro error ~5.4e-3 vs 2e-2 gate).
  outT_aug = v_aug^T @ P^T  accumulated over key tiles; ALL FOUR head
        chains share ONE psum bank (disjoint 17-row groups): a [128,1]
        zero matmul with start=True clears the bank's has_written bits
        once, every chain matmul then runs start=False (first write per
        element overwrites, later ones accumulate).  Row 16 of each
        32-row col-group = sum_j P^T[j, q] (softmax denominator).
  bc   = Sel^T @ outT  broadcasts each group's sum row over the group
  outT_norm = outT * reciprocal_approx_fast(bc)   (custom DVE op, ~5x
        faster than the exact iterative-divide reciprocal)
  partial = outT_norm^T @ Wp_spread   (zero rows kill sum/garbage rows)

PSUM budget (8 banks): 3 x [128,1024] score tiles (6 banks, also reused
as stage-A qkv/v-transpose scratch) + 1 shared attnv accumulator bank +
1 bank for the bc/pp normalize/projection tail (keeping the tail off the
hot score ring removed ~3.3us of PE idle per (nn,g2)).

Matmul dtypes: bf16 throughout the matmul path -- qkv projection
(x/weights bf16, fp32 PSUM accumulation), q/k stores + scores
(HW microbench: a 4-MM row-tiled scores group runs 422ns in bf16 vs
771ns f32r -- FWL fast-weight-load only engages for non-fp32 weights)
and for attnv (a 4-MM col-tiled group measures ~257ns, near-full 4-way
tile_position concurrency, even with per-MM weight churn).

Measured (loop-delta method, 8 cores): 418us baseline -> ~292us.
"""

import numpy as np

import concourse.bass as bass
import concourse.mybir as mybir
import concourse.tile as tile
from concourse import bacc

F32 = mybir.dt.float32
F32R = mybir.dt.float32r
BF16 = mybir.dt.bfloat16
I16 = mybir.dt.int16
EXPF = mybir.ActivationFunctionType.Exp

P = 128
B, N_FULL, C, H, D = 4, 2048, 256, 16, 16
CC = C // P  # 2 channel tiles
NCORES = 8

MM_DT = "bf16"    # qkv projection matmuls + x/weights (FWL weight path)
AV_DT = "bf16"    # attnv matmuls (col-group tile_position needs bf16 weights)
PROJ_DT = "bf16"  # sum-broadcast + output projection matmuls (FWL weight path)

_DT = {"f32r": F32R, "f32": F32, "bf16": BF16}

# Schraudolph int16/bf16 exp constants: bits = round(SC_A*s + SC_B);
# bitcast(bits) ~= exp(s), rel err rms ~1.8%, max ~4.2% (HW-validated).
_LOG2E = 1.4426950408889634
SC_A = 128.0 * _LOG2E
SC_B = 128.0 * 127.0 - 7.4

# Of each 32 consecutive (kt, pr) exp units, this many go to the DVE
# (Schraudolph) instead of ScalarE (table exp). Golden-ratio spread.
DVE_UNITS = 12


def _dve_pattern(n=DVE_UNITS):
    if n == 12:
        # constructed: exactly one DVE unit per kt (never both), pr
        # alternating, the 4 all-ACT kts evenly spaced
        pat = np.zeros(32, bool)
        for kt in range(16):
            if kt % 4 != 3:
                pat[2 * kt + (kt % 2)] = True
        return pat
    # Golden-ratio spread; measured best of the tried patterns (the HW is
    # very sensitive to this assignment: plain golden 12 units or a mod-3
    # pattern both cost +7..+60us).
    idx = np.argsort((np.arange(32) * 0.61803398875) % 1.0)[:n]
    pat = np.zeros(32, bool)
    pat[idx] = True
    return pat


DVE_PAT = _dve_pattern()

USE_DMA_T = True  # x transpose via DMA xbar (4 x 64-partition fp32 tiles)

_NC_CACHE: dict = {}
LAST_RESULT = None  # BassKernelResults of the most recent run (for test.py)
TIMING_REPS = 1  # >1 repeats the compute on-device (timing); output unchanged


def build(n_tokens=N_FULL, reps=1):
    N = n_tokens
    KT = N // P   # key tiles
    QC = 512      # q-chunk (psum bank = 512 fp32)
    NQ = N // QC
    TT = N // P   # token tiles

    MD = _DT[MM_DT]
    AD = _DT[AV_DT]
    PD = _DT[PROJ_DT]

    nc = bacc.Bacc()
    if USE_DMA_T:
        # host passes x already transposed: [C, N]
        x_d = nc.dram_tensor("x", [C, N], MD, kind="ExternalInput")
    else:
        x_d = nc.dram_tensor("x", [N, C], F32, kind="ExternalInput")
    wq_d = nc.dram_tensor("wq", [2, C, P], MD, kind="ExternalInput")
    wk_d = nc.dram_tensor("wk", [2, C, P], MD, kind="ExternalInput")
    wv_d = nc.dram_tensor("wv", [C, P], MD, kind="ExternalInput")
    bq_d = nc.dram_tensor("bq", [2, P], F32, kind="ExternalInput")
    bk_d = nc.dram_tensor("bk", [2, P], F32, kind="ExternalInput")
    bv_d = nc.dram_tensor("bv", [P], F32, kind="ExternalInput")
    wp_d = nc.dram_tensor("wp", [2, P, C], PD, kind="ExternalInput")
    sel_d = nc.dram_tensor("sel", [P, P], PD, kind="ExternalInput")
    idn_d = nc.dram_tensor("idn", [P, P], F32, kind="ExternalInput")
    out_d = nc.dram_tensor("out", [N, C], F32, kind="ExternalOutput")

    with tile.TileContext(nc) as tc:
        with (
            tc.tile_pool(name="const", bufs=1) as const,
            tc.tile_pool(name="work", bufs=6) as work,
            tc.tile_pool(name="ptp", bufs=10) as ptp,
            # One ring of 3 x [128,1024] (2 banks each): sc score tiles plus
            # stage-A psum scratch (qkv, v-transpose) rotate here.
            tc.tile_pool(name="ps_s", bufs=3, space="PSUM") as ps_s,
            # All 4 attnv accumulation chains share ONE bank (disjoint
            # 17-row groups).
            tc.tile_pool(name="ps_at", bufs=1, space="PSUM") as ps_at,
            # bc/pp normalize+projection scratch: keeps the tail off the
            # hot sc ring (those stalls cost ~3.3us per (nn,g2)).
            tc.tile_pool(name="ps_sm", bufs=1, space="PSUM") as ps_sm,
        ):
            ps_m = ps_s  # stage-A scratch shares the scores ring
            # ---------------- loads ----------------
            def staged_load(name, shape, dt, src_ap):
                sb = const.tile(shape, dt, name=f"{name}_sb")
                nc.sync.dma_start(sb[:], src_ap)
                return sb

            # Small constants first: the first qkv matmuls wait on the
            # weights — queuing them behind the 2 MB x transfer costs
            # ~10 us of PE idle at startup.
            idn_sb = staged_load("idn", [P, P], F32, idn_d[:])
            wq_sb = staged_load(
                "wq", [P, 2, CC, P], MD,
                wq_d[:].rearrange("g (cc p) f -> p g cc f", p=P),
            )
            wk_sb = staged_load(
                "wk", [P, 2, CC, P], MD,
                wk_d[:].rearrange("g (cc p) f -> p g cc f", p=P),
            )
            wv_sb = staged_load(
                "wv", [P, CC, P], MD, wv_d[:].rearrange("(cc p) f -> p cc f", p=P)
            )
            if USE_DMA_T:
                # x arrives host-transposed [C, N]: DMA straight into the
                # [channels-on-partitions] layout, no on-chip transposes.
                # Chunk 0 queued before the remaining small constants so
                # the first projections unblock as early as possible.
                xt_full = const.tile([P, CC, N], MD, name="xt_full")
                x_r = x_d[:].rearrange("(cc p) t -> p cc t", p=P)
                for cc in range(CC):
                    nc.sync.dma_start(
                        xt_full[:, cc, 0:QC], x_r[:, cc, 0:QC]
                    )
            bq_sb = staged_load("bq", [P, 2], F32, bq_d[:].rearrange("g p -> p g"))
            bk_sb = staged_load("bk", [P, 2], F32, bk_d[:].rearrange("g p -> p g"))
            bv_sb = staged_load(
                "bv", [P, 1], F32, bv_d[:].rearrange("(p o) -> p o", o=1)
            )
            wp_sb = staged_load("wp", [P, 2, C], PD, wp_d[:].rearrange("g p c -> p g c"))
            sel_sb = staged_load("sel", [P, P], PD, sel_d[:])

            if USE_DMA_T:
                for cc in range(CC):
                    for qq in range(1, NQ):
                        nc.sync.dma_start(
                            xt_full[:, cc, qq * QC : (qq + 1) * QC],
                            x_r[:, cc, qq * QC : (qq + 1) * QC],
                        )
                x_sb = None
            else:
                x_sb = const.tile([P, TT, C], F32)
                x_r = x_d[:].rearrange("(t p) c -> p t c", p=P)
                for tt in range(TT):
                    nc.sync.dma_start(x_sb[:, tt, :], x_r[:, tt, :])
                xt_full = None

            from contextlib import nullcontext

            loop_ctx = tc.For_i(0, reps, 1) if reps > 1 else nullcontext()
            with loop_ctx:
                _build_body(
                    nc, tc, const, work, ptp, ps_s, ps_at, ps_sm,
                    N, KT, QC, NQ, TT, MD, AD, PD,
                    x_sb, xt_full, wq_sb, wk_sb, wv_sb, wp_sb, sel_sb, idn_sb,
                    bq_sb, bk_sb, bv_sb, out_d,
                )
    nc.finalize()
    return nc


def _build_body(
    nc, tc, const, work, ptp, ps_s, ps_at, ps_sm,
    N, KT, QC, NQ, TT, MD, AD, PD,
    x_sb, xt_full, wq_sb, wk_sb, wv_sb, wp_sb, sel_sb, idn_sb,
    bq_sb, bk_sb, bv_sb, out_d,
):
    ps_m = ps_s
    ones_sb = const.tile([P, 1], F32)
    nc.vector.memset(ones_sb[:], 1.0)
    zeros_sb = const.tile([P, 1], F32)
    nc.vector.memset(zeros_sb[:], 0.0)
    # [1,128] zero row for the at-bank has_written clear matmul
    zrow_sb = const.tile([1, P], F32)
    nc.vector.memset(zrow_sb[:], 0.0)
    KC = QC // P  # key tiles per chunk
    # q/k stored bf16: scores matmuls then use the FWL bf16 weight path,
    # ~1.8x faster per row-tiled group than f32r (HW microbench 422 vs 771
    # ns); projection stays f32r-accurate, only the store rounds.
    qt_t = [const.tile([P, 2, QC], AD, name=f"qt{c}") for c in range(NQ)]
    kt_t = [const.tile([P, 2, QC], AD, name=f"kt{c}") for c in range(NQ)]
    vt_t = [const.tile([P, QC], F32, name=f"vt{c}") for c in range(NQ)]
    vaug_t = [
        const.tile([P, KC, 8, 17], AD, name=f"vaug{c}") for c in range(NQ)
    ]
    if xt_full is None:
        xt_t = [const.tile([P, CC, QC], MD, name=f"xt{c}") for c in range(NQ)]
    else:
        xt_t = None

    # ot_raw ping-pong buffers: garbage rows (17..31 of each 32-group)
    # zeroed once here, the 17-row copies below never touch them.
    ot_raw_pp = []
    for i in range(2):
        t = const.tile([P, QC], PD, name=f"otraw{i}")
        nc.vector.tensor_copy(t[:], zeros_sb[:, 0:1].to_broadcast((P, QC)))
        ot_raw_pp.append(t)

    def xt_ap(c):
        if xt_full is not None:
            return xt_full[:, :, c * QC : (c + 1) * QC]
        return xt_t[c][:]

    for c in range(NQ):
        if xt_full is None:
            # xT for this chunk via PE transpose
            for ti in range(QC // P):
                tt = c * (QC // P) + ti
                for cc in range(CC):
                    tp = ps_m.tile([P, P], F32, tag="scores", name="tp")
                    nc.tensor.transpose(
                        tp[:], x_sb[:, tt, cc * P : (cc + 1) * P], idn_sb[:]
                    )
                    nc.vector.tensor_copy(
                        xt_t[c][:, cc, ti * P : (ti + 1) * P], tp[:]
                    )
        xc = xt_ap(c)
        # k, v (needed for all q-chunks) then q projections
        projs = [
            (wk_sb[:, 0], bk_sb[:, 0:1], kt_t[c][:, 0]),
            (wk_sb[:, 1], bk_sb[:, 1:2], kt_t[c][:, 1]),
            (wv_sb[:], bv_sb[:, 0:1], vt_t[c][:]),
            (wq_sb[:, 0], bq_sb[:, 0:1], qt_t[c][:, 0]),
            (wq_sb[:, 1], bq_sb[:, 1:2], qt_t[c][:, 1]),
        ]
        for w_ap, b_ap, dslice in projs:
            ps = ps_m.tile([P, QC], F32, tag="scores", name="ps")
            for cc in range(CC):
                nc.tensor.matmul(
                    ps[:],
                    w_ap[:, cc, :],
                    xc[:, cc, :],
                    start=(cc == 0),
                    stop=(cc == CC - 1),
                )
            # single fused PSUM->SBUF copy + per-partition bias add
            nc.vector.tensor_scalar_add(dslice, ps[:], b_ap)
        # v_aug for this chunk (v natural layout + ones column)
        nc.vector.tensor_copy(
            vaug_t[c][:, :, :, 16],
            ones_sb[:, 0:1, None].to_broadcast((P, KC, 8)),
        )
        for ki in range(KC):
            tp = ps_m.tile([P, P], F32, tag="scores", name="tp")
            nc.tensor.transpose(
                tp[:], vt_t[c][:, ki * P : (ki + 1) * P], idn_sb[:]
            )
            nc.vector.tensor_copy(
                vaug_t[c][:, ki, :, 0:16],
                tp[:].rearrange("p (h d) -> p h d", d=16),
            )

    # ---------------- attention ----------------
    for nn in range(NQ):
        ot_n = work.tile([P, 2, QC], PD, tag="otn")
        for g2 in range(2):
            # All 4 lj accumulation chains share one psum bank (disjoint
            # 17-row groups).  One [128,1] zero matmul with start=True
            # clears the bank's has_written bits; every chain matmul then
            # uses start=False (first write per element overwrites, later
            # ones accumulate).  The [128,1] write overlaps all chains'
            # rows so subtile deps order every chain after the clear.
            at = ps_at.tile([P, QC], F32, tag="at", name="at")
            nc.tensor.matmul(
                at[:, 0:1], zrow_sb[:], ones_sb[0:1, 0:1],
                start=True, stop=True,
            )
            for kt in range(KT):
                scs = []
                for pr in range(2):
                    sc = ps_s.tile([P, 2 * QC], F32, tag="scores", name="sc")
                    for j2 in range(2):
                        lj = 2 * pr + j2
                        rg = 32 * lj
                        nc.tensor.matmul(
                            sc[:, j2 * QC : (j2 + 1) * QC],
                            kt_t[kt // KC][
                                rg : rg + D, g2,
                                (kt % KC) * P : (kt % KC + 1) * P,
                            ],
                            qt_t[nn][rg : rg + D, g2, :],
                            start=True,
                            stop=True,
                            tile_position=(rg, 0),
                        )
                    scs.append(sc)
                pts = []
                for pr in range(2):
                    if DVE_PAT[(2 * kt + pr) % 32]:
                        # DVE Schraudolph exp: affine -> int16 bits == bf16
                        pt = ptp.tile([P, 2 * QC], I16, tag="pt", name="pt")
                        nc.vector.tensor_scalar(
                            pt[:], scs[pr][:], SC_A, SC_B,
                            mybir.AluOpType.mult, mybir.AluOpType.add,
                        )
                        pts.append(pt[:].bitcast(BF16))
                    else:
                        pt = ptp.tile([P, 2 * QC], AD, tag="pt", name="pt")
                        nc.scalar.activation(pt[:], scs[pr][:], EXPF)
                        pts.append(pt[:])
                for pr in range(2):
                    for j2 in range(2):
                        lj = 2 * pr + j2
                        nc.tensor.matmul(
                            at[32 * lj : 32 * lj + 17, :],
                            vaug_t[kt // KC][:, kt % KC, 4 * g2 + lj, :],
                            pts[pr][:, j2 * QC : (j2 + 1) * QC],
                            start=False,
                            stop=(kt == KT - 1),
                            tile_position=(0, 32 * lj),
                        )
            # normalize: broadcast sums over each col-group, divide
            ot_raw = ot_raw_pp[(2 * nn + g2) % 2]
            for lj in range(4):
                nc.vector.tensor_copy(
                    ot_raw[32 * lj : 32 * lj + 17, :],
                    at[32 * lj : 32 * lj + 17, :],
                )
            bc = ps_sm.tile([P, QC], F32, tag="small", name="bc")
            nc.tensor.matmul(
                bc[:], sel_sb[:], ot_raw[:], start=True, stop=True
            )
            rec = work.tile([P, QC], F32, tag="rec")
            nc.vector.reciprocal_approx_fast(rec[:], bc[:])
            nc.vector.tensor_mul(ot_n[:, g2, :], ot_raw[:], rec[:])
        # output projection for this q-chunk
        for ss in range(QC // P):
            pp = ps_sm.tile([P, C], F32, tag="small", name="pp")
            for g2 in range(2):
                nc.tensor.matmul(
                    pp[:],
                    ot_n[:, g2, ss * P : (ss + 1) * P],
                    wp_sb[:, g2, :],
                    start=(g2 == 0),
                    stop=(g2 == 1),
                )
            ob = work.tile([P, C], F32, tag="ob")
            nc.vector.tensor_copy(ob[:], pp[:])
            tt_idx = nn * (QC // P) + ss
            nc.sync.dma_start(
                out_d[:].rearrange("(t p) c -> p t c", p=P)[:, tt_idx, :],
                ob[:],
            )


def _get_nc(n_tokens=N_FULL, reps=1):
    key = (n_tokens, MM_DT, AV_DT, PROJ_DT, DVE_UNITS, USE_DMA_T, reps)
    if key not in _NC_CACHE:
        _NC_CACHE[key] = build(n_tokens, reps=reps)
    return _NC_CACHE[key]


def make_core_inputs(core, x, w_qkv, b_qkv, w_proj, n_tokens=N_FULL):
    """Host-side sharding: slice/spread weights for one core."""
    b, g = core // 2, core % 2
    wq_s = np.zeros((2, C, P), np.float32)
    wk_s = np.zeros((2, C, P), np.float32)
    bq_s = np.zeros((2, P), np.float32)
    bk_s = np.zeros((2, P), np.float32)
    wv_s = np.zeros((C, P), np.float32)
    bv_s = np.zeros((P,), np.float32)
    wp_s = np.zeros((2, P, C), np.float32)
    for g2 in range(2):
        for j in range(4):
            h = 8 * g + 4 * g2 + j
            sp = slice(32 * j, 32 * j + D)
            wq_s[g2, :, sp] = w_qkv[:, 0 * C + h * D : 0 * C + (h + 1) * D]
            wk_s[g2, :, sp] = w_qkv[:, 1 * C + h * D : 1 * C + (h + 1) * D]
            bq_s[g2, sp] = b_qkv[0 * C + h * D : 0 * C + (h + 1) * D]
            bk_s[g2, sp] = b_qkv[1 * C + h * D : 1 * C + (h + 1) * D]
            wp_s[g2, sp, :] = w_proj[h * D : (h + 1) * D, :]
    for lh in range(8):
        h = 8 * g + lh
        wv_s[:, 16 * lh : 16 * lh + 16] = w_qkv[:, 2 * C + h * D : 2 * C + (h + 1) * D]
        bv_s[16 * lh : 16 * lh + 16] = b_qkv[2 * C + h * D : 2 * C + (h + 1) * D]
    sel = np.zeros((P, P), np.float32)
    for j in range(4):
        sel[32 * j + 16, 32 * j : 32 * j + 32] = 1.0
    idn = np.eye(P, dtype=np.float32)

    def cast(a, stage_dt):
        if stage_dt == "bf16":
            import ml_dtypes
            return a.astype(ml_dtypes.bfloat16)
        return a.astype(np.float32)

    if USE_DMA_T:
        x_core = cast(np.ascontiguousarray(x[b, :n_tokens].T), MM_DT)
    else:
        x_core = np.ascontiguousarray(x[b, :n_tokens], dtype=np.float32)
    return {
        "x": x_core,
        "wq": cast(wq_s, MM_DT), "wk": cast(wk_s, MM_DT), "wv": cast(wv_s, MM_DT),
        "bq": bq_s, "bk": bk_s, "bv": bv_s,
        "wp": cast(wp_s, PROJ_DT), "sel": cast(sel, PROJ_DT), "idn": idn,
    }


def kernel(x, w_qkv, b_qkv, w_proj, b_proj):
    global LAST_RESULT
    from concourse.bass_utils import run_bass_kernel_spmd

    x = np.asarray(x, dtype=np.float32)
    w_qkv = np.asarray(w_qkv, dtype=np.float32)
    b_qkv = np.asarray(b_qkv, dtype=np.float32)
    w_proj = np.asarray(w_proj, dtype=np.float32)
    b_proj = np.asarray(b_proj, dtype=np.float32)

    nc = _get_nc(reps=TIMING_REPS)
    in_maps = [
        make_core_inputs(core, x, w_qkv, b_qkv, w_proj) for core in range(NCORES)
    ]
    res = run_bass_kernel_spmd(nc, in_maps, list(range(NCORES)))
    LAST_RESULT = res
    out = np.zeros((B, N_FULL, C), np.float32)
    for core in range(NCORES):
        out[core // 2] += res.results[core]["out"]
    out += b_proj[None, None, :]
    return out


# revision 32
# speedup vs baseline: 1.3157x; 1.3157x over previous
"""Trainium2 Bass kernel for multi-head attention (B=4, N=2048, C=256, H=16).

Sharding: 8 cores, each core handles one batch b = core//2 and 8 heads
(half of 16) g = core%2.  Each core computes its 8 heads' attention plus a
partial output projection (its heads' rows of w_proj); the host sums the
two partials per batch and adds b_proj.  x is transposed on the host so
the [channels-on-partitions] layout DMAs straight in (no on-chip
transposes for x).

Per-core on-chip algorithm (all layouts "transposed", channels on
partitions):
  qT/kT (spread layout: head j of a 4-head group occupies partitions
        32j..32j+16) = W^T @ xT, bias fused into the single PSUM->SBUF
        tensor_scalar_add                                     [128, N]
  vT   (compact: head lh at partitions 16lh)                  [128, N]
  v_aug[keys, lh, 0:16] = v, v_aug[keys, lh, 16] = 1          (ones col
        makes the attn@v matmul also produce softmax row-sums)
  S^T  = k_h @ q_h^T   (row-group-packed matmuls, K=16)       [keys, q]
  P^T  = exp(S^T): the 256 exp units ([128,1024] tiles) are split
        ~2:1 between ScalarE (table exp -> bf16) and VectorE (one fused
        tensor_scalar: bits = round(128*log2e*s + (128*127-7.4)) as
        int16, whose bit pattern IS bf16(exp(s)) -- Schraudolph approx,
        rel err rms ~1.8%; consumed via a free bitcast).  ScalarE alone
        is a ~290us serial bottleneck; the split takes exp off the
        critical path (final rel_fro error ~5.4e-3 vs 2e-2 gate).
  outT_aug = v_aug^T @ P^T  accumulated over key tiles; ALL FOUR head
        chains share ONE psum bank (disjoint 17-row groups): a [128,1]
        zero matmul with start=True clears the bank's has_written bits
        once, every chain matmul then runs start=False (first write per
        element overwrites, later ones accumulate).  Row 16 of each
        32-row col-group = sum_j P^T[j, q] (softmax denominator).
  bc   = Sel^T @ outT  broadcasts each group's sum row over the group
  outT_norm = outT * reciprocal_approx_fast(bc)   (custom DVE op, ~5x
        faster than the exact iterative-divide reciprocal)
  partial = outT_norm^T @ Wp_spread   (zero rows kill sum/garbage rows)

PSUM budget (8 banks): 3 x [128,1024] score tiles (6 banks, also reused
as stage-A qkv/v-transpose scratch) + 1 shared attnv accumulator bank +
1 bank for the bc/pp normalize/projection tail (keeping the tail off the
hot score ring removed ~3.3us of PE idle per (nn,g2)).

Matmul dtypes: bf16 throughout the matmul path -- qkv projection
(x/weights bf16, fp32 PSUM accumulation), q/k stores + scores
(HW microbench: a 4-MM row-tiled scores group runs 422ns in bf16 vs
771ns f32r -- FWL fast-weight-load only engages for non-fp32 weights)
and for attnv (a 4-MM col-tiled group measures ~257ns, near-full 4-way
tile_position concurrency, even with per-MM weight churn).

Measured (loop-delta method, 8 cores): 418us baseline -> ~292us.
"""

import numpy as np

import concourse.bass as bass
import concourse.mybir as mybir
import concourse.tile as tile
from concourse import bacc

F32 = mybir.dt.float32
F32R = mybir.dt.float32r
BF16 = mybir.dt.bfloat16
I16 = mybir.dt.int16
EXPF = mybir.ActivationFunctionType.Exp

P = 128
B, N_FULL, C, H, D = 4, 2048, 256, 16, 16
CC = C // P  # 2 channel tiles
NCORES = 8

MM_DT = "bf16"    # qkv projection matmuls + x/weights (FWL weight path)
AV_DT = "bf16"    # attnv matmuls (col-group tile_position needs bf16 weights)
PROJ_DT = "bf16"  # sum-broadcast + output projection matmuls (FWL weight path)

_DT = {"f32r": F32R, "f32": F32, "bf16": BF16}

# Schraudolph int16/bf16 exp constants: bits = round(SC_A*s + SC_B);
# bitcast(bits) ~= exp(s), rel err rms ~1.8%, max ~4.2% (HW-validated).
_LOG2E = 1.4426950408889634
SC_A = 128.0 * _LOG2E
SC_B = 128.0 * 127.0 - 7.4

# Of each 32 consecutive (kt, pr) exp units, this many go to the DVE
# (Schraudolph) instead of ScalarE (table exp). Golden-ratio spread.
DVE_UNITS = 11


def _dve_pattern(n=DVE_UNITS):
    # Golden-ratio spread; measured best of the tried patterns (the HW is
    # very sensitive to this assignment: 12 units or a mod-3 pattern both
    # cost +7..+60us).
    idx = np.argsort((np.arange(32) * 0.61803398875) % 1.0)[:n]
    pat = np.zeros(32, bool)
    pat[idx] = True
    return pat


DVE_PAT = _dve_pattern()

USE_DMA_T = True  # x transpose via DMA xbar (4 x 64-partition fp32 tiles)

_NC_CACHE: dict = {}
LAST_RESULT = None  # BassKernelResults of the most recent run (for test.py)
TIMING_REPS = 1  # >1 repeats the compute on-device (timing); output unchanged


def build(n_tokens=N_FULL, reps=1):
    N = n_tokens
    KT = N // P   # key tiles
    QC = 512      # q-chunk (psum bank = 512 fp32)
    NQ = N // QC
    TT = N // P   # token tiles

    MD = _DT[MM_DT]
    AD = _DT[AV_DT]
    PD = _DT[PROJ_DT]

    nc = bacc.Bacc()
    if USE_DMA_T:
        # host passes x already transposed: [C, N]
        x_d = nc.dram_tensor("x", [C, N], MD, kind="ExternalInput")
    else:
        x_d = nc.dram_tensor("x", [N, C], F32, kind="ExternalInput")
    wq_d = nc.dram_tensor("wq", [2, C, P], MD, kind="ExternalInput")
    wk_d = nc.dram_tensor("wk", [2, C, P], MD, kind="ExternalInput")
    wv_d = nc.dram_tensor("wv", [C, P], MD, kind="ExternalInput")
    bq_d = nc.dram_tensor("bq", [2, P], F32, kind="ExternalInput")
    bk_d = nc.dram_tensor("bk", [2, P], F32, kind="ExternalInput")
    bv_d = nc.dram_tensor("bv", [P], F32, kind="ExternalInput")
    wp_d = nc.dram_tensor("wp", [2, P, C], PD, kind="ExternalInput")
    sel_d = nc.dram_tensor("sel", [P, P], PD, kind="ExternalInput")
    idn_d = nc.dram_tensor("idn", [P, P], F32, kind="ExternalInput")
    out_d = nc.dram_tensor("out", [N, C], F32, kind="ExternalOutput")

    with tile.TileContext(nc) as tc:
        with (
            tc.tile_pool(name="const", bufs=1) as const,
            tc.tile_pool(name="work", bufs=6) as work,
            tc.tile_pool(name="ptp", bufs=10) as ptp,
            # One ring of 3 x [128,1024] (2 banks each): sc score tiles plus
            # stage-A psum scratch (qkv, v-transpose) rotate here.
            tc.tile_pool(name="ps_s", bufs=3, space="PSUM") as ps_s,
            # All 4 attnv accumulation chains share ONE bank (disjoint
            # 17-row groups).
            tc.tile_pool(name="ps_at", bufs=1, space="PSUM") as ps_at,
            # bc/pp normalize+projection scratch: keeps the tail off the
            # hot sc ring (those stalls cost ~3.3us per (nn,g2)).
            tc.tile_pool(name="ps_sm", bufs=1, space="PSUM") as ps_sm,
        ):
            ps_m = ps_s  # stage-A scratch shares the scores ring
            # ---------------- loads ----------------
            def staged_load(name, shape, dt, src_ap):
                sb = const.tile(shape, dt, name=f"{name}_sb")
                nc.sync.dma_start(sb[:], src_ap)
                return sb

            # Small constants first: the first qkv matmuls wait on the
            # weights — queuing them behind the 2 MB x transfer costs
            # ~10 us of PE idle at startup.
            idn_sb = staged_load("idn", [P, P], F32, idn_d[:])
            wq_sb = staged_load(
                "wq", [P, 2, CC, P], MD,
                wq_d[:].rearrange("g (cc p) f -> p g cc f", p=P),
            )
            wk_sb = staged_load(
                "wk", [P, 2, CC, P], MD,
                wk_d[:].rearrange("g (cc p) f -> p g cc f", p=P),
            )
            wv_sb = staged_load(
                "wv", [P, CC, P], MD, wv_d[:].rearrange("(cc p) f -> p cc f", p=P)
            )
            if USE_DMA_T:
                # x arrives host-transposed [C, N]: DMA straight into the
                # [channels-on-partitions] layout, no on-chip transposes.
                # Chunk 0 queued before the remaining small constants so
                # the first projections unblock as early as possible.
                xt_full = const.tile([P, CC, N], MD, name="xt_full")
                x_r = x_d[:].rearrange("(cc p) t -> p cc t", p=P)
                for cc in range(CC):
                    nc.sync.dma_start(
                        xt_full[:, cc, 0:QC], x_r[:, cc, 0:QC]
                    )
            bq_sb = staged_load("bq", [P, 2], F32, bq_d[:].rearrange("g p -> p g"))
            bk_sb = staged_load("bk", [P, 2], F32, bk_d[:].rearrange("g p -> p g"))
            bv_sb = staged_load(
                "bv", [P, 1], F32, bv_d[:].rearrange("(p o) -> p o", o=1)
            )
            wp_sb = staged_load("wp", [P, 2, C], PD, wp_d[:].rearrange("g p c -> p g c"))
            sel_sb = staged_load("sel", [P, P], PD, sel_d[:])

            if USE_DMA_T:
                for cc in range(CC):
                    for qq in range(1, NQ):
                        nc.sync.dma_start(
                            xt_full[:, cc, qq * QC : (qq + 1) * QC],
                            x_r[:, cc, qq * QC : (qq + 1) * QC],
                        )
                x_sb = None
            else:
                x_sb = const.tile([P, TT, C], F32)
                x_r = x_d[:].rearrange("(t p) c -> p t c", p=P)
                for tt in range(TT):
                    nc.sync.dma_start(x_sb[:, tt, :], x_r[:, tt, :])
                xt_full = None

            from contextlib import nullcontext

            loop_ctx = tc.For_i(0, reps, 1) if reps > 1 else nullcontext()
            with loop_ctx:
                _build_body(
                    nc, tc, const, work, ptp, ps_s, ps_at, ps_sm,
                    N, KT, QC, NQ, TT, MD, AD, PD,
                    x_sb, xt_full, wq_sb, wk_sb, wv_sb, wp_sb, sel_sb, idn_sb,
                    bq_sb, bk_sb, bv_sb, out_d,
                )
    nc.finalize()
    return nc


def _build_body(
    nc, tc, const, work, ptp, ps_s, ps_at, ps_sm,
    N, KT, QC, NQ, TT, MD, AD, PD,
    x_sb, xt_full, wq_sb, wk_sb, wv_sb, wp_sb, sel_sb, idn_sb,
    bq_sb, bk_sb, bv_sb, out_d,
):
    ps_m = ps_s
    ones_sb = const.tile([P, 1], F32)
    nc.vector.memset(ones_sb[:], 1.0)
    zeros_sb = const.tile([P, 1], F32)
    nc.vector.memset(zeros_sb[:], 0.0)
    # [1,128] zero row for the at-bank has_written clear matmul
    zrow_sb = const.tile([1, P], F32)
    nc.vector.memset(zrow_sb[:], 0.0)
    KC = QC // P  # key tiles per chunk
    # q/k stored bf16: scores matmuls then use the FWL bf16 weight path,
    # ~1.8x faster per row-tiled group than f32r (HW microbench 422 vs 771
    # ns); projection stays f32r-accurate, only the store rounds.
    qt_t = [const.tile([P, 2, QC], AD, name=f"qt{c}") for c in range(NQ)]
    kt_t = [const.tile([P, 2, QC], AD, name=f"kt{c}") for c in range(NQ)]
    vt_t = [const.tile([P, QC], F32, name=f"vt{c}") for c in range(NQ)]
    vaug_t = [
        const.tile([P, KC, 8, 17], AD, name=f"vaug{c}") for c in range(NQ)
    ]
    if xt_full is None:
        xt_t = [const.tile([P, CC, QC], MD, name=f"xt{c}") for c in range(NQ)]
    else:
        xt_t = None

    # ot_raw ping-pong buffers: garbage rows (17..31 of each 32-group)
    # zeroed once here, the 17-row copies below never touch them.
    ot_raw_pp = []
    for i in range(2):
        t = const.tile([P, QC], PD, name=f"otraw{i}")
        nc.vector.tensor_copy(t[:], zeros_sb[:, 0:1].to_broadcast((P, QC)))
        ot_raw_pp.append(t)

    def xt_ap(c):
        if xt_full is not None:
            return xt_full[:, :, c * QC : (c + 1) * QC]
        return xt_t[c][:]

    for c in range(NQ):
        if xt_full is None:
            # xT for this chunk via PE transpose
            for ti in range(QC // P):
                tt = c * (QC // P) + ti
                for cc in range(CC):
                    tp = ps_m.tile([P, P], F32, tag="scores", name="tp")
                    nc.tensor.transpose(
                        tp[:], x_sb[:, tt, cc * P : (cc + 1) * P], idn_sb[:]
                    )
                    nc.vector.tensor_copy(
                        xt_t[c][:, cc, ti * P : (ti + 1) * P], tp[:]
                    )
        xc = xt_ap(c)
        # k, v (needed for all q-chunks) then q projections
        projs = [
            (wk_sb[:, 0], bk_sb[:, 0:1], kt_t[c][:, 0]),
            (wk_sb[:, 1], bk_sb[:, 1:2], kt_t[c][:, 1]),
            (wv_sb[:], bv_sb[:, 0:1], vt_t[c][:]),
            (wq_sb[:, 0], bq_sb[:, 0:1], qt_t[c][:, 0]),
            (wq_sb[:, 1], bq_sb[:, 1:2], qt_t[c][:, 1]),
        ]
        for w_ap, b_ap, dslice in projs:
            ps = ps_m.tile([P, QC], F32, tag="scores", name="ps")
            for cc in range(CC):
                nc.tensor.matmul(
                    ps[:],
                    w_ap[:, cc, :],
                    xc[:, cc, :],
                    start=(cc == 0),
                    stop=(cc == CC - 1),
                )
            # single fused PSUM->SBUF copy + per-partition bias add
            nc.vector.tensor_scalar_add(dslice, ps[:], b_ap)
        # v_aug for this chunk (v natural layout + ones column)
        nc.vector.tensor_copy(
            vaug_t[c][:, :, :, 16],
            ones_sb[:, 0:1, None].to_broadcast((P, KC, 8)),
        )
        for ki in range(KC):
            tp = ps_m.tile([P, P], F32, tag="scores", name="tp")
            nc.tensor.transpose(
                tp[:], vt_t[c][:, ki * P : (ki + 1) * P], idn_sb[:]
            )
            nc.vector.tensor_copy(
                vaug_t[c][:, ki, :, 0:16],
                tp[:].rearrange("p (h d) -> p h d", d=16),
            )

    # ---------------- attention ----------------
    for nn in range(NQ):
        ot_n = work.tile([P, 2, QC], PD, tag="otn")
        for g2 in range(2):
            # All 4 lj accumulation chains share one psum bank (disjoint
            # 17-row groups).  One [128,1] zero matmul with start=True
            # clears the bank's has_written bits; every chain matmul then
            # uses start=False (first write per element overwrites, later
            # ones accumulate).  The [128,1] write overlaps all chains'
            # rows so subtile deps order every chain after the clear.
            at = ps_at.tile([P, QC], F32, tag="at", name="at")
            nc.tensor.matmul(
                at[:, 0:1], zrow_sb[:], ones_sb[0:1, 0:1],
                start=True, stop=True,
            )
            for kt in range(KT):
                scs = []
                for pr in range(2):
                    sc = ps_s.tile([P, 2 * QC], F32, tag="scores", name="sc")
                    for j2 in range(2):
                        lj = 2 * pr + j2
                        rg = 32 * lj
                        nc.tensor.matmul(
                            sc[:, j2 * QC : (j2 + 1) * QC],
                            kt_t[kt // KC][
                                rg : rg + D, g2,
                                (kt % KC) * P : (kt % KC + 1) * P,
                            ],
                            qt_t[nn][rg : rg + D, g2, :],
                            start=True,
                            stop=True,
                            tile_position=(rg, 0),
                        )
                    scs.append(sc)
                pts = []
                for pr in range(2):
                    if DVE_PAT[(2 * kt + pr) % 32]:
                        # DVE Schraudolph exp: affine -> int16 bits == bf16
                        pt = ptp.tile([P, 2 * QC], I16, tag="pt", name="pt")
                        nc.vector.tensor_scalar(
                            pt[:], scs[pr][:], SC_A, SC_B,
                            mybir.AluOpType.mult, mybir.AluOpType.add,
                        )
                        pts.append(pt[:].bitcast(BF16))
                    else:
                        pt = ptp.tile([P, 2 * QC], AD, tag="pt", name="pt")
                        nc.scalar.activation(pt[:], scs[pr][:], EXPF)
                        pts.append(pt[:])
                for pr in range(2):
                    for j2 in range(2):
                        lj = 2 * pr + j2
                        nc.tensor.matmul(
                            at[32 * lj : 32 * lj + 17, :],
                            vaug_t[kt // KC][:, kt % KC, 4 * g2 + lj, :],
                            pts[pr][:, j2 * QC : (j2 + 1) * QC],
                            start=False,
                            stop=(kt == KT - 1),
                            tile_position=(0, 32 * lj),
                        )
            # normalize: broadcast sums over each col-group, divide
            ot_raw = ot_raw_pp[(2 * nn + g2) % 2]
            for lj in range(4):
                nc.vector.tensor_copy(
                    ot_raw[32 * lj : 32 * lj + 17, :],
                    at[32 * lj : 32 * lj + 17, :],
                )
            bc = ps_sm.tile([P, QC], F32, tag="small", name="bc")
            nc.tensor.matmul(
                bc[:], sel_sb[:], ot_raw[:], start=True, stop=True
            )
            rec = work.tile([P, QC], F32, tag="rec")
            nc.vector.reciprocal_approx_fast(rec[:], bc[:])
            nc.vector.tensor_mul(ot_n[:, g2, :], ot_raw[:], rec[:])
        # output projection for this q-chunk
        for ss in range(QC // P):
            pp = ps_sm.tile([P, C], F32, tag="small", name="pp")
            for g2 in range(2):
                nc.tensor.matmul(
                    pp[:],
                    ot_n[:, g2, ss * P : (ss + 1) * P],
                    wp_sb[:, g2, :],
                    start=(g2 == 0),
                    stop=(g2 == 1),
                )
            ob = work.tile([P, C], F32, tag="ob")
            nc.vector.tensor_copy(ob[:], pp[:])
            tt_idx = nn * (QC // P) + ss
            nc.sync.dma_start(
                out_d[:].rearrange("(t p) c -> p t c", p=P)[:, tt_idx, :],
                ob[:],
            )


def _get_nc(n_tokens=N_FULL, reps=1):
    key = (n_tokens, MM_DT, AV_DT, PROJ_DT, DVE_UNITS, USE_DMA_T, reps)
    if key not in _NC_CACHE:
        _NC_CACHE[key] = build(n_tokens, reps=reps)
    return _NC_CACHE[key]


def make_core_inputs(core, x, w_qkv, b_qkv, w_proj, n_tokens=N_FULL):
    """Host-side sharding: slice/spread weights for one core."""
    b, g = core // 2, core % 2
    wq_s = np.zeros((2, C, P), np.float32)
    wk_s = np.zeros((2, C, P), np.float32)
    bq_s = np.zeros((2, P), np.float32)
    bk_s = np.zeros((2, P), np.float32)
    wv_s = np.zeros((C, P), np.float32)
    bv_s = np.zeros((P,), np.float32)
    wp_s = np.zeros((2, P, C), np.float32)
    for g2 in range(2):
        for j in range(4):
            h = 8 * g + 4 * g2 + j
            sp = slice(32 * j, 32 * j + D)
            wq_s[g2, :, sp] = w_qkv[:, 0 * C + h * D : 0 * C + (h + 1) * D]
            wk_s[g2, :, sp] = w_qkv[:, 1 * C + h * D : 1 * C + (h + 1) * D]
            bq_s[g2, sp] = b_qkv[0 * C + h * D : 0 * C + (h + 1) * D]
            bk_s[g2, sp] = b_qkv[1 * C + h * D : 1 * C + (h + 1) * D]
            wp_s[g2, sp, :] = w_proj[h * D : (h + 1) * D, :]
    for lh in range(8):
        h = 8 * g + lh
        wv_s[:, 16 * lh : 16 * lh + 16] = w_qkv[:, 2 * C + h * D : 2 * C + (h + 1) * D]
        bv_s[16 * lh : 16 * lh + 16] = b_qkv[2 * C + h * D : 2 * C + (h + 1) * D]
    sel = np.zeros((P, P), np.float32)
    for j in range(4):
        sel[32 * j + 16, 32 * j : 32 * j + 32] = 1.0
    idn = np.eye(P, dtype=np.float32)

    def cast(a, stage_dt):
        if stage_dt == "bf16":
            import ml_dtypes
            return a.astype(ml_dtypes.bfloat16)
        return a.astype(np.float32)

    if USE_DMA_T:
        x_core = cast(np.ascontiguousarray(x[b, :n_tokens].T), MM_DT)
    else:
        x_core = np.ascontiguousarray(x[b, :n_tokens], dtype=np.float32)
    return {
        "x": x_core,
        "wq": cast(wq_s, MM_DT), "wk": cast(wk_s, MM_DT), "wv": cast(wv_s, MM_DT),
        "bq": bq_s, "bk": bk_s, "bv": bv_s,
        "wp": cast(wp_s, PROJ_DT), "sel": cast(sel, PROJ_DT), "idn": idn,
    }


def kernel(x, w_qkv, b_qkv, w_proj, b_proj):
    global LAST_RESULT
    from concourse.bass_utils import run_bass_kernel_spmd

    x = np.asarray(x, dtype=np.float32)
    w_qkv = np.asarray(w_qkv, dtype=np.float32)
    b_qkv = np.asarray(b_qkv, dtype=np.float32)
    w_proj = np.asarray(w_proj, dtype=np.float32)
    b_proj = np.asarray(b_proj, dtype=np.float32)

    nc = _get_nc(reps=TIMING_REPS)
    in_maps = [
        make_core_inputs(core, x, w_qkv, b_qkv, w_proj) for core in range(NCORES)
    ]
    res = run_bass_kernel_spmd(nc, in_maps, list(range(NCORES)))
    LAST_RESULT = res
    out = np.zeros((B, N_FULL, C), np.float32)
    for core in range(NCORES):
        out[core // 2] += res.results[core]["out"]
    out += b_proj[None, None, :]
    return out
